# revision 1
# baseline (speedup 1.0000x reference)
"""ArcFace loss on 8 TRN2 NeuronCores — class-parallel (vocab-sharded).

Math: loss = mean_b[ M0 + ln(Z'_b) - s*phi_b ] with
  Z_b  = sum_c exp(s*cos(b,c) - M0)          (device, sharded over classes)
  Z'_b = Z_b - exp(s*cos(b,l_b) - M0) + exp(s*phi_b - M0)   (label correction)
M0 is a fixed logsumexp shift: |cos| <= 1 so s*cos - M0 <= 80 and
exp() can never overflow f32 (e^80 < f32 max); on the real data
|s*cos| <= ~36, so all terms stay in the normal f32 range.

Host (inside kernel()): row-normalize x and W, fold nothing into W, transpose
W shards to [D, C_shard] so the device needs no on-chip transposes, and
evaluate the tiny label/phi terms ([512] vectors). Device: the 512x512x100k
matmul, exp, row-sums, an AllGather of per-core partial Z, and the final
scalar reduction.
"""

import math

import numpy as np

from concourse import bacc, mybir
from concourse.bass_utils import run_bass_kernel_spmd
from concourse.tile import TileContext

NCORES = 8
B = 512
D = 512
C = 100000
CS = 12544  # per-core classes, padded: 8 * 12544 = 100352 >= C
S = 120.0
MARGIN = 0.3
COS_M = math.cos(MARGIN)
SIN_M = math.sin(MARGIN)
TH = math.cos(math.pi - MARGIN)
MM = math.sin(math.pi - MARGIN) * MARGIN
M0 = 40.0  # logsumexp shift
SUPER = 2048  # class columns per DMA (1 MiB per [128, 2048] f32 tile)
NBLK = 512  # class columns per matmul (one PSUM bank)

F32 = mybir.dt.float32
BF16 = mybir.dt.bfloat16
FN = mybir.ActivationFunctionType

_GRAPH = None
LAST_RESULT = None  # BassKernelResults of the most recent run (for test harness)


def _build_nc(repeat=1):
    """Build the SPMD graph. repeat>1 unrolls the whole body N times into one
    NEFF (timing only: amortizes the per-execute dispatch overhead)."""
    nc = bacc.Bacc("TRN2", target_bir_lowering=False)

    # const AP for the Exp bias (only 0.0/1.0 are pre-registered)
    _cb = nc.alloc_sbuf_tensor(f"const-float32-{-M0}", [128, 1], F32)
    nc.gpsimd.memset(_cb.ap(), -M0)
    nc.const_aps.aps[(F32, -M0)] = _cb.ap()
    nc.all_engine_barrier()

    xt = nc.declare_dram_parameter("xt", [D, B], BF16, isOutput=False)
    wt = nc.declare_dram_parameter("wt", [D, CS], BF16, isOutput=False)
    sl = nc.declare_dram_parameter("sl", [1, B], F32, isOutput=False)
    su = nc.declare_dram_parameter("su", [1, B], F32, isOutput=False)
    out = nc.declare_dram_parameter("out", [1, 1], F32, isOutput=True)

    with TileContext(nc, num_cores=NCORES) as tc:
        with (
            tc.tile_pool(name="xpool", bufs=1) as xpool,
            tc.tile_pool(name="wpool", bufs=3) as wpool,
            tc.tile_pool(name="epool", bufs=3) as epool,
            tc.tile_pool(name="zpool", bufs=1) as zpool,
            tc.tile_pool(name="spool", bufs=1) as spool,
            tc.tile_pool(name="psum", bufs=4, space="PSUM") as pp,
            tc.tile_pool(name="dram", bufs=1, space="DRAM") as dram,
        ):
            # x^T (normalized, transposed on host): 4 contraction chunks [128, B]
            xts = []
            for k in range(4):
                t = xpool.tile([128, B], BF16, tag=f"xt{k}", name=f"xts{k}")
                nc.sync.dma_start(t[:], xt[k * 128 : (k + 1) * 128, :])
                xts.append(t)

            sl_sb = spool.tile([1, B], F32, tag="sl")
            nc.sync.dma_start(sl_sb[:], sl[:])
            su_sb = spool.tile([1, B], F32, tag="su")
            nc.sync.dma_start(su_sb[:], su[:])

            # label-correction terms: computed up front, hidden under main loop
            t1 = spool.tile([1, B], F32, tag="t1")
            nc.scalar.activation(t1[:], sl_sb[:], FN.Exp, bias=-M0, scale=1.0)
            t2 = spool.tile([1, B], F32, tag="t2")
            nc.scalar.activation(t2[:], su_sb[:], FN.Exp, bias=-M0, scale=1.0)

            for rep in range(repeat):
                _body(nc, tc, rep, xpool, wpool, epool, zpool, spool, pp, dram,
                      xt, wt, out, xts, sl_sb, su_sb, t1, t2)

    if not nc.is_finalized():
        nc.finalize()
    return nc


def _body(nc, tc, rep, xpool, wpool, epool, zpool, spool, pp, dram,
          xt, wt, out, xts, sl_sb, su_sb, t1, t2):
    # per-batch-tile accumulators of per-block exp-sums (one col/block)
    zbufs = [
        zpool.tile([128, 32], F32, tag=f"zb{bi}", name=f"zb{bi}_{rep}")
        for bi in range(4)
    ]

    # superblock widths: small leading chunks (incl. the ragged 256
    # remainder) sit in the DMA ramp shadow where the PE is waiting anyway;
    # the steady-state tail is uniform full-width superblocks
    sws = [256, 512, 512, 1024] + [SUPER] * ((CS - 2304) // SUPER)
    assert sum(sws) == CS and all(w > 0 for w in sws)

    col = 0
    c0 = 0
    for sw in sws:
        wts = []
        for k in range(4):
            t = wpool.tile([128, SUPER], BF16, tag=f"w{k}", name=f"wts{k}_{rep}")
            nc.sync.dma_start(
                t[:, :sw], wt[k * 128 : (k + 1) * 128, c0 : c0 + sw]
            )
            wts.append(t)
        for bi in range(4):
          for h0 in range(0, sw, 1024):
            hw_ = min(1024, sw - h0)
            ps = pp.tile([128, 1024], F32, tag="ps", name=f"ps_{rep}")
            nb0 = 0
            while nb0 < hw_:
                nb = min(NBLK, hw_ - nb0)
                for k in range(4):
                    nc.tensor.matmul(
                        ps[:, nb0 : nb0 + nb],
                        xts[k][:, bi * 128 : (bi + 1) * 128],
                        wts[k][:, h0 + nb0 : h0 + nb0 + nb],
                        start=(k == 0),
                        stop=(k == 3),
                    )
                nb0 += nb
            ex = epool.tile([128, 1024], F32, tag="ex", name=f"ex_{rep}")
            nc.scalar.activation(
                ex[:, :hw_],
                ps[:, :hw_],
                FN.Exp,
                bias=-M0,
                scale=S,
                accum_out=zbufs[bi][:, col + h0 // 1024 : col + h0 // 1024 + 1],
            )
        col += (sw + 1023) // 1024
        c0 += sw
    ncol = col  # number of superblocks

    # partial Z per core -> DRAM [B] in interleaved order b' = p*4 + bi
    # (one [128,4] DMA instead of four partition-gather DMAs; the host
    # permutes sl/su to the same order, and the final mean is order-blind)
    zdram = dram.tile([B], F32, name=f"zdram_{rep}")
    zs_all = zpool.tile([128, 4], F32, tag="zsall", name=f"zsall_{rep}")
    for bi in range(4):
        nc.vector.reduce_sum(
            zs_all[:, bi : bi + 1], zbufs[bi][:, :ncol],
            axis=mybir.AxisListType.X,
        )
    nc.sync.dma_start(zdram[:], zs_all[:])

    zgat = dram.tile([NCORES * B], F32, name=f"zgat_{rep}")
    nc.gpsimd.collective_compute(
        "AllGather",
        mybir.AluOpType.bypass,
        replica_groups=[list(range(NCORES))],
        ins=[zdram.opt()],
        outs=[zgat.opt()],
    )

    # cross-core sum of the gathered partials on the PE:
    # zsum[1, B] = ones[1, 8] @ zg[8, B]
    zg = spool.tile([NCORES, B], F32, tag="zg", name=f"zg_{rep}")
    nc.sync.dma_start(zg[:], zgat.rearrange("(r b) -> r b", r=NCORES))
    ones = nc.const_aps.aps[(F32, 1.0)]
    zps = pp.tile([128, 1024], F32, tag="ps", name=f"zps_{rep}")
    nc.tensor.matmul(
        zps[:1, :B], ones[:NCORES, :1], zg[:], start=True, stop=True
    )

    # label correction + final scalar
    zc = spool.tile([1, B], F32, tag="zc", name=f"zc_{rep}")
    nc.vector.tensor_sub(zc[:], zps[:1, :B], t1[:])
    nc.vector.tensor_add(zc[:], zc[:], t2[:])
    lg = spool.tile([1, B], F32, tag="lg", name=f"lg_{rep}")
    nc.scalar.activation(lg[:], zc[:], FN.Ln)
    v = spool.tile([1, B], F32, tag="v", name=f"v_{rep}")
    nc.vector.tensor_sub(v[:], lg[:], su_sb[:])
    r = spool.tile([1, 1], F32, tag="r", name=f"r_{rep}")
    nc.vector.reduce_sum(r[:], v[:], axis=mybir.AxisListType.X)
    ov = spool.tile([1, 1], F32, tag="ov", name=f"ov_{rep}")
    nc.scalar.activation(ov[:], r[:], FN.Copy, bias=M0, scale=1.0 / B)
    nc.sync.dma_start(out[:], ov[:])


def _host_prep(input, label, weight):
    x = np.asarray(input, dtype=np.float32)
    lab = np.asarray(label).astype(np.int64).ravel()
    w = np.asarray(weight, dtype=np.float32)

    xn64 = x.astype(np.float64)
    xn64 /= np.maximum(
        np.sqrt(np.einsum("bd,bd->b", xn64, xn64))[:, None], 1e-12
    )
    bf16 = mybir.dt.np(BF16)
    xt = np.ascontiguousarray(xn64.T.astype(np.float32)).astype(bf16)  # [D, B]

    wn_inv = 1.0 / np.maximum(
        np.sqrt(np.einsum("cd,cd->c", w, w, dtype=np.float64)), 1e-12
    )
    wn = w * wn_inv[:, None].astype(np.float32)  # [C, D] normalized rows, f32

    # label terms (tiny, f64)
    wl = wn[lab].astype(np.float64)  # [B, D]
    cosl = np.einsum("bd,bd->b", xn64, wl)
    cosl = np.clip(cosl, -1.0, 1.0)
    sine = np.sqrt(np.maximum(1.0 - cosl * cosl, 0.0))
    phi = cosl * COS_M - sine * SIN_M
    phi = np.where(cosl > TH, phi, cosl - MM)
    # device z lands in interleaved order b' = p*4 + bi (batch b = bi*128+p);
    # permute the label-term vectors to match
    perm = (np.arange(B) % 4) * 128 + np.arange(B) // 4
    sl = (S * cosl)[perm].astype(np.float32).reshape(1, B)
    su = (S * phi)[perm].astype(np.float32).reshape(1, B)

    # class-sharded, transposed W: [D, CS] per core, zero-padded at the tail
    shards = []
    for i in range(NCORES):
        lo, hi = i * CS, min((i + 1) * CS, C)
        sh = np.zeros((D, CS), dtype=bf16)
        sh[:, : hi - lo] = wn[lo:hi].T.astype(bf16)
        shards.append(np.ascontiguousarray(sh))
    return xt, sl, su, shards


def kernel(input, label, weight):
    global _GRAPH, LAST_RESULT
    xt, sl, su, shards = _host_prep(input, label, weight)
    if _GRAPH is None:
        _GRAPH = _build_nc()
    in_maps = [
        {"xt": xt, "wt": shards[i], "sl": sl, "su": su} for i in range(NCORES)
    ]
    res = run_bass_kernel_spmd(_GRAPH, in_maps, list(range(NCORES)))
    LAST_RESULT = res
    outv = np.asarray(res.results[0]["out"], dtype=np.float32)
    return outv.reshape(())



# revision 2
# speedup vs baseline: 1.0003x; 1.0003x over previous
"""ArcFace loss on 8 TRN2 NeuronCores — class-parallel (vocab-sharded), fp8.

Math: loss = mean_b[ M0 + ln(Z'_b) - s*phi_b ] with
  Z_b  = sum_c exp(s*cos(b,c) - M0)          (device, sharded over classes)
  Z'_b = Z_b - exp(s*cosq(b,l_b) - M0) + exp(s*phi_b - M0)
cosq is the device's fp8 cosine for the label class (host replicates the
fp8 dot exactly so the correction cancels the device term); phi uses the
exact f64 cosine. M0 is a fixed logsumexp shift: |cos| <= ~1.07 even with
fp8 rounding, so s*cos - M0 <= 89 and exp() never overflows f32.

Device: x and W rows are unit-normalized on host, scaled by 32 and
quantized to fp8 e4m3 (max |elem| = 32 << 448), so the 512x512x100k
matmul runs in DoubleRow perf mode (2 fp8 weights per PE cell, two
128-row contraction chunks per instruction). The Exp runs on the scalar
engine in 2048-wide instructions reading PSUM, writing bf16 to SBUF
without accum_out; the idle vector engine does the row-sum reduction.
Then an AllGather of per-core partial Z and a tiny scalar tail.
"""

import math

import numpy as np

from concourse import bacc, mybir
from concourse.bass_utils import run_bass_kernel_spmd
from concourse.tile import TileContext

NCORES = 8
B = 512
D = 512
C = 100000
CS = 12544  # per-core classes, padded: 8 * 12544 = 100352 >= C
S = 120.0
MARGIN = 0.3
COS_M = math.cos(MARGIN)
SIN_M = math.sin(MARGIN)
TH = math.cos(math.pi - MARGIN)
MM = math.sin(math.pi - MARGIN) * MARGIN
M0 = 40.0  # logsumexp shift
QS = 32.0  # fp8 quantization scale for x and W (unit rows -> |elem*QS| <= 32)
SUPER = 2048  # class columns per superblock (one Exp instruction)
NBLK = 512  # class columns per matmul (one PSUM bank)
SBS = [SUPER] * 6 + [256]  # superblock widths; sum == CS
assert sum(SBS) == CS

F32 = mybir.dt.float32
BF16 = mybir.dt.bfloat16
F8 = mybir.dt.float8e4
FN = mybir.ActivationFunctionType
DR = mybir.MatmulPerfMode.DoubleRow

_GRAPH = None
LAST_RESULT = None  # BassKernelResults of the most recent run (for test harness)


def _build_nc(repeat=1):
    """Build the SPMD graph. repeat>1 unrolls the whole body N times into one
    NEFF (timing only: amortizes the per-execute dispatch overhead)."""
    nc = bacc.Bacc("TRN2", target_bir_lowering=False)

    # const AP for the Exp bias (only 0.0/1.0 are pre-registered)
    _cb = nc.alloc_sbuf_tensor(f"const-float32-{-M0}", [128, 1], F32)
    nc.gpsimd.memset(_cb.ap(), -M0)
    nc.const_aps.aps[(F32, -M0)] = _cb.ap()
    nc.all_engine_barrier()

    # x^T fp8, DoubleRow pairs: row kp*128+p, col i*B+b = x[b, (2kp+i)*128+p]
    xt = nc.declare_dram_parameter("xt", [256, 2 * B], F8, isOutput=False)
    # W^T fp8, DoubleRow pairs, superblock-major: per pair row-block and
    # superblock (c0, sw), cols [2*c0 : 2*c0+2*sw] hold [2, sw] row-major
    wt = nc.declare_dram_parameter("wt", [256, 2 * CS], F8, isOutput=False)
    sl = nc.declare_dram_parameter("sl", [1, B], F32, isOutput=False)
    su = nc.declare_dram_parameter("su", [1, B], F32, isOutput=False)
    out = nc.declare_dram_parameter("out", [1, 1], F32, isOutput=True)

    with TileContext(nc, num_cores=NCORES) as tc:
        with (
            tc.tile_pool(name="xpool", bufs=1) as xpool,
            tc.tile_pool(name="wpool", bufs=3) as wpool,
            tc.tile_pool(name="epool", bufs=3) as epool,
            tc.tile_pool(name="zpool", bufs=1) as zpool,
            tc.tile_pool(name="spool", bufs=1) as spool,
            tc.tile_pool(name="psum", bufs=2, space="PSUM") as pp,
            tc.tile_pool(name="dram", bufs=1, space="DRAM") as dram,
        ):
            # x^T fp8 pair tiles [K=128, sub=2, B]
            xts = []
            for kp in range(2):
                t = xpool.tile([128, 2, B], F8, tag=f"xt{kp}", name=f"xts{kp}")
                nc.sync.dma_start(
                    t[:],
                    xt[kp * 128 : (kp + 1) * 128, :].rearrange(
                        "p (s b) -> p s b", s=2
                    ),
                )
                xts.append(t)

            sl_sb = spool.tile([1, B], F32, tag="sl")
            nc.sync.dma_start(sl_sb[:], sl[:])
            su_sb = spool.tile([1, B], F32, tag="su")
            nc.sync.dma_start(su_sb[:], su[:])

            # label-correction terms: computed up front, hidden under main loop
            t1 = spool.tile([1, B], F32, tag="t1")
            nc.scalar.activation(t1[:], sl_sb[:], FN.Exp, bias=-M0, scale=1.0)
            t2 = spool.tile([1, B], F32, tag="t2")
            nc.scalar.activation(t2[:], su_sb[:], FN.Exp, bias=-M0, scale=1.0)

            for rep in range(repeat):
                _body(nc, tc, rep, xpool, wpool, epool, zpool, spool, pp, dram,
                      xt, wt, out, xts, sl_sb, su_sb, t1, t2)

    if not nc.is_finalized():
        nc.finalize()
    return nc


def _body(nc, tc, rep, xpool, wpool, epool, zpool, spool, pp, dram,
          xt, wt, out, xts, sl_sb, su_sb, t1, t2):
    # per-batch-tile partial exp-sums, one col per superblock
    zbufs = [
        zpool.tile([128, 8], F32, tag=f"zb{bi}", name=f"zb{bi}_{rep}")
        for bi in range(4)
    ]

    c0 = 0
    for sbi, sw in enumerate(SBS):
        wts = []
        for kp in range(2):
            t = wpool.tile(
                [128, 2, SUPER], F8, tag=f"w{kp}", name=f"wts{kp}_{rep}"
            )
            nc.sync.dma_start(
                t[:, :, :sw],
                wt[
                    kp * 128 : (kp + 1) * 128, 2 * c0 : 2 * c0 + 2 * sw
                ].rearrange("p (s c) -> p s c", s=2),
            )
            wts.append(t)
        for bi in range(4):
            ps = pp.tile([128, SUPER], F32, tag="ps", name=f"ps_{rep}")
            for nb0 in range(0, sw, NBLK):
                nb = min(NBLK, sw - nb0)
                for kp in range(2):
                    nc.tensor.matmul(
                        ps[:, nb0 : nb0 + nb],
                        xts[kp][:, :, bi * 128 : (bi + 1) * 128],
                        wts[kp][:, :, nb0 : nb0 + nb],
                        start=(kp == 0),
                        stop=(kp == 1),
                        perf_mode=DR,
                    )
            ex = epool.tile([128, SUPER], BF16, tag="ex", name=f"ex_{rep}")
            nc.scalar.activation(
                ex[:, :sw], ps[:, :sw], FN.Exp, bias=-M0, scale=S / (QS * QS)
            )
            nc.vector.reduce_sum(
                zbufs[bi][:, sbi : sbi + 1], ex[:, :sw],
                axis=mybir.AxisListType.X,
            )
        c0 += sw

    # partial Z per core -> DRAM [B] in interleaved order b' = p*4 + bi
    # (one [128,4] DMA instead of four partition-gather DMAs; the host
    # permutes sl/su to the same order, and the final mean is order-blind)
    zdram = dram.tile([B], F32, name=f"zdram_{rep}")
    zs_all = zpool.tile([128, 4], F32, tag="zsall", name=f"zsall_{rep}")
    for bi in range(4):
        nc.vector.reduce_sum(
            zs_all[:, bi : bi + 1], zbufs[bi][:, : len(SBS)],
            axis=mybir.AxisListType.X,
        )
    nc.sync.dma_start(zdram[:], zs_all[:])

    zgat = dram.tile([NCORES * B], F32, name=f"zgat_{rep}")
    nc.gpsimd.collective_compute(
        "AllGather",
        mybir.AluOpType.bypass,
        replica_groups=[list(range(NCORES))],
        ins=[zdram.opt()],
        outs=[zgat.opt()],
    )

    # cross-core sum of the gathered partials on the PE:
    # zsum[1, B] = ones[1, 8] @ zg[8, B]
    zg = spool.tile([NCORES, B], F32, tag="zg", name=f"zg_{rep}")
    nc.sync.dma_start(zg[:], zgat.rearrange("(r b) -> r b", r=NCORES))
    ones = nc.const_aps.aps[(F32, 1.0)]
    zps = pp.tile([128, SUPER], F32, tag="ps", name=f"zps_{rep}")
    nc.tensor.matmul(
        zps[:1, :B], ones[:NCORES, :1], zg[:], start=True, stop=True
    )

    # label correction + final scalar
    zc = spool.tile([1, B], F32, tag="zc", name=f"zc_{rep}")
    nc.vector.tensor_sub(zc[:], zps[:1, :B], t1[:])
    nc.vector.tensor_add(zc[:], zc[:], t2[:])
    lg = spool.tile([1, B], F32, tag="lg", name=f"lg_{rep}")
    nc.scalar.activation(lg[:], zc[:], FN.Ln)
    v = spool.tile([1, B], F32, tag="v", name=f"v_{rep}")
    nc.vector.tensor_sub(v[:], lg[:], su_sb[:])
    r = spool.tile([1, 1], F32, tag="r", name=f"r_{rep}")
    nc.vector.reduce_sum(r[:], v[:], axis=mybir.AxisListType.X)
    ov = spool.tile([1, 1], F32, tag="ov", name=f"ov_{rep}")
    nc.scalar.activation(ov[:], r[:], FN.Copy, bias=M0, scale=1.0 / B)
    nc.sync.dma_start(out[:], ov[:])


def _dr_pack(aT):
    """[D, N] (D=512) -> [256, 2*N]: DoubleRow pair layout. Row kp*128+p,
    col i*N+n = aT[(2*kp+i)*128 + p, n]."""
    d, n = aT.shape
    chunks = aT.reshape(4, 128, n)
    pairs = [
        np.stack([chunks[2 * kp], chunks[2 * kp + 1]], axis=1).reshape(
            128, 2 * n
        )
        for kp in range(2)
    ]
    return np.concatenate(pairs, axis=0)


def _host_prep(input, label, weight):
    x = np.asarray(input, dtype=np.float32)
    lab = np.asarray(label).astype(np.int64).ravel()
    w = np.asarray(weight, dtype=np.float32)
    f8 = mybir.dt.np(F8)

    xn64 = x.astype(np.float64)
    xn64 /= np.maximum(
        np.sqrt(np.einsum("bd,bd->b", xn64, xn64))[:, None], 1e-12
    )
    xq = (xn64 * QS).astype(np.float32).astype(f8)  # [B, D] fp8
    xt = np.ascontiguousarray(_dr_pack(xq.astype(np.float32).T).astype(f8))

    wn_inv = 1.0 / np.maximum(
        np.sqrt(np.einsum("cd,cd->c", w, w, dtype=np.float64)), 1e-12
    )
    wn = w * wn_inv[:, None].astype(np.float32)  # [C, D] normalized rows, f32
    wq = (wn * QS).astype(f8)  # [C, D] fp8

    # label terms (tiny): phi from the exact f64 cosine, the Z-correction
    # from the fp8 cosine the device actually summed
    wl = wn[lab].astype(np.float64)  # [B, D]
    cosl = np.einsum("bd,bd->b", xn64, wl)
    cosl = np.clip(cosl, -1.0, 1.0)
    sine = np.sqrt(np.maximum(1.0 - cosl * cosl, 0.0))
    phi = cosl * COS_M - sine * SIN_M
    phi = np.where(cosl > TH, phi, cosl - MM)
    cosq = np.einsum(
        "bd,bd->b",
        xq.astype(np.float32),
        wq[lab].astype(np.float32),
        dtype=np.float64,
    ) / (QS * QS)
    # device z lands in interleaved order b' = p*4 + bi (batch b = bi*128+p);
    # permute the label-term vectors to match
    perm = (np.arange(B) % 4) * 128 + np.arange(B) // 4
    sl = (S * cosq)[perm].astype(np.float32).reshape(1, B)
    su = (S * phi)[perm].astype(np.float32).reshape(1, B)

    # class-sharded, transposed, DoubleRow-packed, superblock-major W
    shards = []
    for i in range(NCORES):
        lo, hi = i * CS, min((i + 1) * CS, C)
        sh = np.zeros((CS, D), dtype=f8)
        sh[: hi - lo] = wq[lo:hi]
        packed = _dr_pack(sh.astype(np.float32).T)  # [256, 2*CS], pair layout
        # rearrange cols to superblock-major [2, sw] blocks
        dst = np.empty_like(packed)
        q = 0
        c0 = 0
        for sw in SBS:
            blk = packed.reshape(256, 2, CS)[:, :, c0 : c0 + sw]
            dst[:, q : q + 2 * sw] = blk.reshape(256, 2 * sw)
            q += 2 * sw
            c0 += sw
        shards.append(np.ascontiguousarray(dst.astype(f8)))
    return xt, sl, su, shards


def kernel(input, label, weight):
    global _GRAPH, LAST_RESULT
    xt, sl, su, shards = _host_prep(input, label, weight)
    if _GRAPH is None:
        _GRAPH = _build_nc()
    in_maps = [
        {"xt": xt, "wt": shards[i], "sl": sl, "su": su} for i in range(NCORES)
    ]
    res = run_bass_kernel_spmd(_GRAPH, in_maps, list(range(NCORES)))
    LAST_RESULT = res
    outv = np.asarray(res.results[0]["out"], dtype=np.float32)
    return outv.reshape(())
